# revision 19
# baseline (speedup 1.0000x reference)
"""Trainium2 Bass kernel for nn_Attn_40046275068166.

Tiny causal MHA over huge batch: x[B=65536, T=34, D=6], 2 heads, head_dim 3.
Pure data parallelism over 8 cores; batch on the 128 SBUF partitions inside
each core; per-example compute in the free dims.

v2 design notes (calibrated on HW probes):
- DVE tensor_tensor runs 2x ONLY when every operand is 2-byte, innermost
  stride +-1, 4B-aligned start, EVEN run length. Odd 17-runs degrade to
  ~0.8 ns/elem. So score planes are padded to J=36 columns and split at
  j=18; all fp16 ops use even runs.
- tensor_reduce is always 1x (~1.05 ns/elem) -> fold rows with 2x fp16
  adds (36->18->(8+2)) before reducing.
- GPSIMD tensor ops share SBUF ports with DVE and starve it 4.5x ->
  gpsimd does DMA orchestration ONLY.
- ACT (scalar engine) runs ~0.87 ns/elem on any strides; it materializes
  the xp broadcast planes (xpb) and the x transpose (xt16) so every DVE
  mul is all-fp16 stride-1, and computes exp.
- Weights are compile-time -> folded q/k projection A = Wq^T Wk / sqrt(hd);
  mask additive -30000 (fp16-safe, exp -> 0) covers the causal upper
  triangles AND the j=34,35 pad columns.

Pipeline: Pool streams x in / res out (SWDGE, parity-split semaphores);
ACT preps tile n+1 (xt16/xpb) then exps tile n; DVE does phase_a(n)
(proj+scores) then phase_b(n-1) (den/PV/out-projection).
"""

import math
from contextlib import ExitStack
from functools import lru_cache

import numpy as np

import concourse.bass as bass
from concourse import mybir
from concourse.bass_utils import run_bass_kernel_spmd

NCORES = 8
T = 34
J = 36              # padded score-plane width
H = 17              # i split
JH = 18             # j split
D = 6
NH = 2
HD = 3
POS = 3
P = 128

F32 = mybir.dt.float32
F16 = mybir.dt.float16

# fp16 constants layout (element offsets)
OFFH_MA = 0                 # maskA [17][18]
OFFH_MC = H * JH            # maskC [17][18]
OFFH_WO = 2 * H * JH        # WoM replicated [6][34][6] (WoM[dm][t][e]=Wo[dm][e])
OFFH_A2 = OFFH_WO + D * T * D   # A2 replicated [2][3(b)][6(hc)][34(t)]
HLEN = OFFH_A2 + 2 * POS * D * T

MASKV = -30000.0


def _ap(t, off, dims):
    p0 = t[:].ap[0]
    return bass.AP(tensor=t, offset=off, ap=[list(p0)] + [list(d) for d in dims])


def build_kernel(bc, G):
    assert bc % (P * G) == 0
    NT = bc // (P * G)

    nc = bass.Bass("TRN2")
    x = nc.dram_tensor("x", [bc, T, D], F32, kind="ExternalInput")
    wth = nc.dram_tensor("wth", [HLEN], F16, kind="ExternalInput")
    out = nc.dram_tensor("out", [bc, T, D], F32, kind="ExternalOutput")

    xr = x[:].rearrange("(n g p) t d -> n p g t d", g=G, p=P)
    outr = out[:].rearrange("(n g p) t d -> n p g t d", g=G, p=P)
    wth_b = bass.AP(tensor=wth, offset=0, ap=[[0, P], [1, HLEN]])

    # element strides
    XIN_SET = G * T * D         # 408
    XIN_G = T * D               # 34*6
    XT_SET = G * D * T          # xt16 [set][g][d][t]
    XT_G = D * T
    XPB_SET = POS * G * T * JH  # xpb [set][a][g][i][j<18] (C reuses cols 0:18)
    XPB_A = G * T * JH
    XPB_G = T * JH
    YV_SET = 2 * G * D * J      # yv [set][w][g][hc][j]
    YV_W = G * D * J
    YV_G = D * J
    TS_SET = G * NH * T * J     # t0/t1 [set][g][h][i][j]
    TS_G = NH * T * J
    TS_H = T * J
    PP_G = NH * T * J           # pp [g][h][i][j]
    PP_H = T * J
    ED_G = NH * H * JH          # ed [g][h][i'][j']
    ED_H = H * JH
    DEN_G = NH * T
    O2_G = T * D
    PR_G = D * T * D            # prod [g][dm][t][e]
    RES_SET = G * T * D

    with ExitStack() as ctx:
        sb = lambda nm, shape, dt=F32: ctx.enter_context(
            nc.sbuf_tensor(nm, shape, dt))
        wsh = sb("wsh", [P, HLEN], F16)
        xin = sb("xin", [P, 2, G, T, D])
        xt16 = sb("xt16", [P, 2, G, D, T], F16)
        xpb = sb("xpb", [P, 2, POS, G, T, JH], F16)
        yv = sb("yv", [P, 2, 2, G, D, J], F16)
        t0 = sb("t0", [P, 2, G, NH, T, J], F16)
        t1 = sb("t1", [P, 2, G, NH, T, J], F16)
        tmp = sb("tmp", [P, G, D, J], F16)
        ed = sb("ed", [P, G, NH, H, JH], F16)
        pp = sb("pp", [P, G, NH, T, J], F16)
        den = sb("den", [P, G, NH, T])
        rcp = sb("rcp", [P, G, NH, T])
        o2 = sb("o2", [P, G, T, D])
        o2b = sb("o2b", [P, G, T, D], F16)
        prod = sb("prod", [P, G, D, T, D], F16)
        res = sb("res", [P, 2, G, T, D])

        sem_names = ["dma_in0", "dma_in1", "const", "xin_done", "prep_done",
                     "s_done", "e_done", "b_done", "res_done", "out0", "out1"]
        sems = {k: ctx.enter_context(nc.semaphore(name=k)) for k in sem_names}

        block = ctx.enter_context(nc.Block())

        @block.gpsimd
        def _(sync):
            # DMA orchestration ONLY (gpsimd tensor ops would starve DVE's
            # SBUF ports). SWDGE: one +16 completion inc per dma_start.
            def store(k):
                sp = k % 2
                sync.wait_ge(sems["res_done"], k + 1)
                sync.dma_start(
                    out=outr[k],
                    in_=_ap(res, sp * RES_SET, [(XIN_G, G), (1, T * D)]),
                ).then_inc(sems["out0" if sp == 0 else "out1"], 16)

            sync.dma_start(out=wsh[:], in_=wth_b).then_inc(sems["const"], 16)
            for n in range(NT):
                s = n % 2
                if n >= 2:
                    sync.wait_ge(sems["xin_done"], n - 1)
                sync.dma_start(
                    out=_ap(xin, s * XIN_SET, [(XIN_G, G), (1, T * D)]),
                    in_=xr[n],
                ).then_inc(sems["dma_in0" if s == 0 else "dma_in1"], 16)
                if n >= 2:
                    store(n - 2)
            store(NT - 2)
            store(NT - 1)
            sync.wait_ge(sems["out0"], 16 * ((NT + 1) // 2))
            sync.wait_ge(sems["out1"], 16 * (NT // 2))

        @block.scalar
        def _(scalar):
            Copy = mybir.ActivationFunctionType.Copy
            Exp = mybir.ActivationFunctionType.Exp

            def prep(n):
                s = n % 2
                scalar.wait_ge(sems["dma_in0" if s == 0 else "dma_in1"],
                               16 * (n // 2 + 1))
                # xt16[g][d][t] = x[g][t][d]
                scalar.activation(
                    out=_ap(xt16, s * XT_SET, [(XT_G, G), (T, D), (1, T)]),
                    in_=_ap(xin, s * XIN_SET, [(XIN_G, G), (1, D), (D, T)]),
                    func=Copy)
                # xpb[a][g][i][j] = xp[g][i][a]  (AB: all i, j<18; C: i,j>=split)
                for a in range(POS):
                    act = scalar.activation(
                        out=_ap(xpb, s * XPB_SET + a * XPB_A,
                                [(XPB_G, G), (JH, T), (1, JH)]),
                        in_=_ap(xin, s * XIN_SET + POS + a,
                                [(XIN_G, G), (D, T), (0, JH)]),
                        func=Copy)
                    if a == POS - 1:
                        act.then_inc(sems["xin_done"], 1)

            def expf(n):
                s = n % 2
                scalar.wait_ge(sems["s_done"], n + 1)
                if n >= 2:
                    scalar.wait_ge(sems["b_done"], n - 1)
                # AB: all i, j<18 ((g,h) merged: TS_G == NH*TS_H)
                scalar.activation(
                    out=_ap(t1, s * TS_SET, [(TS_H, G * NH), (J, T), (1, JH)]),
                    in_=_ap(t0, s * TS_SET, [(TS_H, G * NH), (J, T), (1, JH)]),
                    func=Exp)
                # C: i>=17, j>=18
                scalar.activation(
                    out=_ap(t1, s * TS_SET + H * J + JH,
                            [(TS_H, G * NH), (J, H), (1, JH)]),
                    in_=_ap(t0, s * TS_SET + H * J + JH,
                            [(TS_H, G * NH), (J, H), (1, JH)]),
                    func=Exp,
                ).then_inc(sems["e_done"], 1)

            scalar.wait_ge(sems["const"], 16)
            prep(0)
            prep(1)
            for n in range(NT):
                expf(n)
                if n + 2 < NT:
                    prep(n + 2)

        @block.vector
        def _(vector):
            vector.wait_ge(sems["const"], 16)
            # zero the j=34,35 pad columns of yv once (proj never writes
            # them; PV/scores read them; mask kills their contribution but
            # they must be finite)
            vector.memset(
                _ap(yv, T, [(J, 2 * 2 * G * D), (1, J - T)]), 0.0)

            def phase_a(n):
                s = n % 2
                # xin_done is incremented by ACT prep(n)'s last copy, which
                # also certifies xt16/xpb for this tile
                vector.wait_ge(sems["xin_done"], n + 1)
                # projections: yv[w][g][hc][j] = sum_b xt16[g][3(1-w)+b][j] * A2[w][hc][b]
                for w in range(2):
                    for b in range(POS):
                        i0 = _ap(xt16, s * XT_SET + (3 * (1 - w) + b) * T,
                                 [(XT_G, G), (0, D), (1, T)])
                        i1 = _ap(wsh, OFFH_A2 + (w * POS + b) * D * T,
                                 [(0, G), (T, D), (1, T)])
                        if b == 0:
                            vector.tensor_mul(
                                out=_ap(yv, s * YV_SET + w * YV_W,
                                        [(YV_G, G), (J, D), (1, T)]),
                                in0=i0, in1=i1)
                        else:
                            vector.tensor_mul(
                                out=_ap(tmp, 0, [(D * J, G), (J, D), (1, T)]),
                                in0=i0, in1=i1)
                            vector.tensor_add(
                                out=_ap(yv, s * YV_SET + w * YV_W,
                                        [(YV_G, G), (J, D), (1, T)]),
                                in0=_ap(yv, s * YV_SET + w * YV_W,
                                        [(YV_G, G), (J, D), (1, T)]),
                                in1=_ap(tmp, 0, [(D * J, G), (J, D), (1, T)]))
                # scores (smul split per head: xpb has no h dim to merge)
                def smul(a, dst):
                    for h in range(NH):
                        # AB: all i, j<18
                        vector.tensor_mul(
                            out=_ap(dst, s * TS_SET + h * TS_H,
                                    [(TS_G, G), (J, T), (1, JH)]),
                            in0=_ap(xpb, s * XPB_SET + a * XPB_A,
                                    [(XPB_G, G), (JH, T), (1, JH)]),
                            in1=_ap(yv, s * YV_SET + (h * HD + a) * J,
                                    [(YV_G, G), (0, T), (1, JH)]))
                        # C: i>=17, j>=18 (xpb cols 0:18 hold the same xp)
                        vector.tensor_mul(
                            out=_ap(dst, s * TS_SET + h * TS_H + H * J + JH,
                                    [(TS_G, G), (J, H), (1, JH)]),
                            in0=_ap(xpb, s * XPB_SET + a * XPB_A + H * JH,
                                    [(XPB_G, G), (JH, H), (1, JH)]),
                            in1=_ap(yv, s * YV_SET + (h * HD + a) * J + JH,
                                    [(YV_G, G), (0, H), (1, JH)]))

                def tadd(region):
                    li = T if region == "AB" else H
                    off = 0 if region == "AB" else H * J + JH
                    vector.tensor_add(
                        out=_ap(t0, s * TS_SET + off,
                                [(TS_H, G * NH), (J, li), (1, JH)]),
                        in0=_ap(t0, s * TS_SET + off,
                                [(TS_H, G * NH), (J, li), (1, JH)]),
                        in1=_ap(t1, s * TS_SET + off,
                                [(TS_H, G * NH), (J, li), (1, JH)]))

                smul(0, t0)
                smul(1, t1)
                tadd("AB")
                tadd("C")
                smul(2, t1)
                # mask: t1[i<17, j<18] += maskA ; t1[C] += maskC
                vector.tensor_add(
                    out=_ap(t1, s * TS_SET, [(TS_H, G * NH), (J, H), (1, JH)]),
                    in0=_ap(t1, s * TS_SET, [(TS_H, G * NH), (J, H), (1, JH)]),
                    in1=_ap(wsh, OFFH_MA, [(0, G * NH), (JH, H), (1, JH)]))
                vector.tensor_add(
                    out=_ap(t1, s * TS_SET + H * J + JH,
                            [(TS_H, G * NH), (J, H), (1, JH)]),
                    in0=_ap(t1, s * TS_SET + H * J + JH,
                            [(TS_H, G * NH), (J, H), (1, JH)]),
                    in1=_ap(wsh, OFFH_MC, [(0, G * NH), (JH, H), (1, JH)]))
                tadd("AB")
                vector.tensor_add(
                    out=_ap(t0, s * TS_SET + H * J + JH,
                            [(TS_H, G * NH), (J, H), (1, JH)]),
                    in0=_ap(t0, s * TS_SET + H * J + JH,
                            [(TS_H, G * NH), (J, H), (1, JH)]),
                    in1=_ap(t1, s * TS_SET + H * J + JH,
                            [(TS_H, G * NH), (J, H), (1, JH)])
                ).then_inc(sems["s_done"], 1)

            def fold_chain(tsrc, soff, rows, into_ed):
                """rows i>=17: fold j[18:36) into j[0:18) (into ed or in
                place), then [16:18)->[0:2), [8:16)->[0:8)."""
                pass  # structured inline below for clarity

            def phase_b(n):
                s = n % 2
                vector.wait_ge(sems["e_done"], n + 1)
                if n >= 2:
                    vector.wait_ge(sems["out0" if s == 0 else "out1"],
                                   16 * (n // 2))
                # den: ed = e[i>=17, j<18] + e[i>=17, j>=18]  ((g,h) merged)
                vector.tensor_add(
                    out=_ap(ed, 0, [(ED_H, G * NH), (JH, H), (1, JH)]),
                    in0=_ap(t1, s * TS_SET + H * J,
                            [(TS_H, G * NH), (J, H), (1, JH)]),
                    in1=_ap(t1, s * TS_SET + H * J + JH,
                            [(TS_H, G * NH), (J, H), (1, JH)]))
                # fold [10:18) onto [0:8), then reduce the contiguous [0:10)
                vector.tensor_add(
                    out=_ap(ed, 0, [(ED_H, G * NH), (JH, H), (1, 8)]),
                    in0=_ap(ed, 0, [(ED_H, G * NH), (JH, H), (1, 8)]),
                    in1=_ap(ed, 10, [(ED_H, G * NH), (JH, H), (1, 8)]))
                vector.tensor_reduce(
                    out=_ap(den, 0, [(T, G * NH), (1, H)]),
                    in_=_ap(t1, s * TS_SET,
                            [(TS_H, G * NH), (J, H), (1, JH)]),
                    axis=mybir.AxisListType.X, op=mybir.AluOpType.add)
                vector.tensor_add(
                    out=_ap(ed, 0, [(ED_H, G * NH), (JH, H), (1, 4)]),
                    in0=_ap(ed, 0, [(ED_H, G * NH), (JH, H), (1, 4)]),
                    in1=_ap(ed, 6, [(ED_H, G * NH), (JH, H), (1, 4)]))
                vector.tensor_reduce(
                    out=_ap(den, H, [(T, G * NH), (1, H)]),
                    in_=_ap(ed, 0, [(ED_H, G * NH), (JH, H), (1, 6)]),
                    axis=mybir.AxisListType.X, op=mybir.AluOpType.add)
                vector.reciprocal(
                    out=_ap(rcp, 0, [(1, G * NH * T)]),
                    in_=_ap(den, 0, [(1, G * NH * T)]))
                # PV per channel c  (yv (g,h) merge: YV_G == NH*HD*J)
                for c in range(HD):
                    vector.tensor_mul(
                        out=_ap(pp, 0, [(PP_H, G * NH), (J, T), (1, JH)]),
                        in0=_ap(t1, s * TS_SET,
                                [(TS_H, G * NH), (J, T), (1, JH)]),
                        in1=_ap(yv, s * YV_SET + YV_W + c * J,
                                [(HD * J, G * NH), (0, T), (1, JH)]))
                    mm = vector.tensor_mul(
                        out=_ap(pp, H * J + JH,
                                [(PP_H, G * NH), (J, H), (1, JH)]),
                        in0=_ap(t1, s * TS_SET + H * J + JH,
                                [(TS_H, G * NH), (J, H), (1, JH)]),
                        in1=_ap(yv, s * YV_SET + YV_W + c * J + JH,
                                [(HD * J, G * NH), (0, H), (1, JH)]))
                    if c == HD - 1:
                        mm.then_inc(sems["b_done"], 1)
                    vector.tensor_add(
                        out=_ap(pp, H * J, [(PP_H, G * NH), (J, H), (1, JH)]),
                        in0=_ap(pp, H * J, [(PP_H, G * NH), (J, H), (1, JH)]),
                        in1=_ap(pp, H * J + JH,
                                [(PP_H, G * NH), (J, H), (1, JH)]))
                    vector.tensor_add(
                        out=_ap(pp, 0, [(PP_H, G * NH), (J, T), (1, 8)]),
                        in0=_ap(pp, 0, [(PP_H, G * NH), (J, T), (1, 8)]),
                        in1=_ap(pp, 10, [(PP_H, G * NH), (J, T), (1, 8)]))
                    vector.tensor_add(
                        out=_ap(pp, 0, [(PP_H, G * NH), (J, T), (1, 4)]),
                        in0=_ap(pp, 0, [(PP_H, G * NH), (J, T), (1, 4)]),
                        in1=_ap(pp, 6, [(PP_H, G * NH), (J, T), (1, 4)]))
                    vector.tensor_reduce(
                        out=_ap(o2, c, [(O2_G, G), (HD, NH), (D, T)]),
                        in_=_ap(pp, 0, [(PP_H, G * NH), (J, T), (1, 6)]),
                        axis=mybir.AxisListType.X, op=mybir.AluOpType.add)
                # normalize -> fp16 (split per head: rcp not (h,c)-mergeable)
                for h in range(NH):
                    vector.tensor_mul(
                        out=_ap(o2b, h * HD,
                                [(O2_G, G), (D, T), (1, HD)]),
                        in0=_ap(o2, h * HD,
                                [(O2_G, G), (D, T), (1, HD)]),
                        in1=_ap(rcp, h * T,
                                [(DEN_G, G), (1, T), (0, HD)]))
                # output projection (WoM replicated over t -> (t,e) merge)
                vector.tensor_mul(
                    out=_ap(prod, 0, [(PR_G, G), (T * D, D), (1, T * D)]),
                    in0=_ap(o2b, 0, [(O2_G, G), (0, D), (1, T * D)]),
                    in1=_ap(wsh, OFFH_WO, [(0, G), (T * D, D), (1, T * D)]))
                vector.tensor_reduce(
                    out=_ap(res, s * RES_SET, [(RES_SET // G, G), (1, D), (D, T)]),
                    in_=_ap(prod, 0, [(PR_G, G), (D, D * T), (1, D)]),
                    axis=mybir.AxisListType.X, op=mybir.AluOpType.add
                ).then_inc(sems["res_done"], 1)

            for n in range(NT):
                phase_a(n)
                if n >= 1:
                    phase_b(n - 1)
            phase_b(NT - 1)

    return nc


def _pack_weights(Wq, Wk, Wv, Wo):
    wth = np.zeros(HLEN, dtype=np.float16)
    scale = 1.0 / math.sqrt(HD)
    A2 = np.zeros((2, D, POS), dtype=np.float64)
    for h in range(NH):
        A2[0, h * HD:(h + 1) * HD, :] = (Wq[h * HD:(h + 1) * HD, :].T
                                         @ Wk[h * HD:(h + 1) * HD, :]) * scale
        A2[1, h * HD:(h + 1) * HD, :] = Wv[h * HD:(h + 1) * HD, :]
    # A2 replicated over t: [w][b][hc][t] = A2[w][hc][b]
    a2r = np.broadcast_to(
        A2.astype(np.float16).transpose(0, 2, 1)[:, :, :, None],
        (2, POS, D, T))
    wth[OFFH_A2:OFFH_A2 + 2 * POS * D * T] = a2r.reshape(-1)
    # WoM replicated over t: [dm][t][e] = Wo[dm][e]
    wom = np.broadcast_to(Wo.astype(np.float16)[:, None, :], (D, T, D))
    wth[OFFH_WO:OFFH_WO + D * T * D] = wom.reshape(-1)
    # maskA: rows i<17, cols j<18: -30000 where j > i
    ma = np.zeros((H, JH), dtype=np.float16)
    for i in range(H):
        ma[i, i + 1:] = MASKV
    # maskC: rows i'=i-17, cols j'=j-18: -30000 where j' >= i'  (covers pad)
    mc = np.zeros((H, JH), dtype=np.float16)
    for i in range(H):
        mc[i, i:] = MASKV
    wth[OFFH_MA:OFFH_MA + H * JH] = ma.reshape(-1)
    wth[OFFH_MC:OFFH_MC + H * JH] = mc.reshape(-1)
    return wth


@lru_cache(maxsize=2)
def _cached_kernel(bc, G):
    return build_kernel(bc, G)


def _prepare(x, Wq, Wk, Wv, Wo, G=4):
    x = np.ascontiguousarray(x, dtype=np.float32)
    B = x.shape[0]
    bc = B // NCORES
    nc = _cached_kernel(bc, G)
    wth = _pack_weights(np.asarray(Wq, dtype=np.float32),
                        np.asarray(Wk, dtype=np.float32),
                        np.asarray(Wv, dtype=np.float32),
                        np.asarray(Wo, dtype=np.float32))
    in_maps = [{"x": x[i * bc:(i + 1) * bc], "wth": wth}
               for i in range(NCORES)]
    return nc, in_maps


def kernel(x, Wq, Wk, Wv, Wo):
    nc, in_maps = _prepare(x, Wq, Wk, Wv, Wo)
    r = run_bass_kernel_spmd(nc, in_maps, core_ids=list(range(NCORES)))
    return np.concatenate([m["out"] for m in r.results], axis=0)


# revision 20
# speedup vs baseline: 1.1779x; 1.1779x over previous
"""Trainium2 Bass kernel for nn_Attn_40046275068166.

Tiny causal MHA over huge batch: x[B=65536, T=34, D=6], 2 heads, head_dim 3.
Pure data parallelism over 8 cores; batch on the 128 SBUF partitions inside
each core; per-example compute in the free dims.

v2 design notes (calibrated on HW probes):
- DVE tensor_tensor runs 2x ONLY when every operand is 2-byte, innermost
  stride +-1, 4B-aligned start, EVEN run length. Odd 17-runs degrade to
  ~0.8 ns/elem. So score planes are padded to J=36 columns and split at
  j=18; all fp16 ops use even runs.
- tensor_reduce is always 1x (~1.05 ns/elem) -> fold rows with 2x fp16
  adds (36->18->(8+2)) before reducing.
- GPSIMD tensor ops share SBUF ports with DVE and starve it 4.5x ->
  gpsimd does DMA orchestration ONLY.
- ACT (scalar engine) runs ~0.87 ns/elem on any strides; it materializes
  the xp broadcast planes (xpb) and the x transpose (xt16) so every DVE
  mul is all-fp16 stride-1, and computes exp.
- Weights are compile-time -> folded q/k projection A = Wq^T Wk / sqrt(hd);
  mask additive -30000 (fp16-safe, exp -> 0) covers the causal upper
  triangles AND the j=34,35 pad columns.

Pipeline: Pool streams x in / res out (SWDGE, parity-split semaphores);
ACT preps tile n+1 (xt16/xpb) then exps tile n; DVE does phase_a(n)
(proj+scores) then phase_b(n-1) (den/PV/out-projection).
"""

import math
from contextlib import ExitStack
from functools import lru_cache

import numpy as np

import concourse.bass as bass
from concourse import mybir
from concourse.bass_utils import run_bass_kernel_spmd

NCORES = 8
T = 34
J = 36              # padded score-plane width
H = 17              # i split
JH = 18             # j split
D = 6
NH = 2
HD = 3
POS = 3
P = 128

F32 = mybir.dt.float32
F16 = mybir.dt.float16

# fp16 constants layout (element offsets)
OFFH_MA = 0                 # maskA [17][18]
OFFH_MC = H * JH            # maskC [17][18]
OFFH_WO = 2 * H * JH        # WoM replicated [6][34][6] (WoM[dm][t][e]=Wo[dm][e])
OFFH_A2 = OFFH_WO + D * T * D   # A2 replicated [2][3(b)][6(hc)][34(t)]
HLEN = OFFH_A2 + 2 * POS * D * T

MASKV = -30000.0


def _ap(t, off, dims):
    p0 = t[:].ap[0]
    return bass.AP(tensor=t, offset=off, ap=[list(p0)] + [list(d) for d in dims])


def build_kernel(bc, G):
    assert bc % (P * G) == 0
    NT = bc // (P * G)

    nc = bass.Bass("TRN2")
    x = nc.dram_tensor("x", [bc, T, D], F32, kind="ExternalInput")
    wth = nc.dram_tensor("wth", [HLEN], F16, kind="ExternalInput")
    out = nc.dram_tensor("out", [bc, T, D], F32, kind="ExternalOutput")

    xr = x[:].rearrange("(n g p) t d -> n p g t d", g=G, p=P)
    outr = out[:].rearrange("(n g p) t d -> n p g t d", g=G, p=P)
    wth_b = bass.AP(tensor=wth, offset=0, ap=[[0, P], [1, HLEN]])

    # element strides
    XIN_SET = G * T * D         # 408
    XIN_G = T * D               # 34*6
    XT_SET = G * D * T          # xt16 [set][g][d][t]
    XT_G = D * T
    XPB_SET = POS * G * T * JH  # xpb [set][a][g][i][j<18] (C reuses cols 0:18)
    XPB_A = G * T * JH
    XPB_G = T * JH
    YV_SET = 2 * G * D * J      # yv [set][w][g][hc][j]
    YV_W = G * D * J
    YV_G = D * J
    TS_SET = G * NH * T * J     # t0/t1 [set][g][h][i][j]
    TS_G = NH * T * J
    TS_H = T * J
    PP_G = NH * T * J           # pp [g][h][i][j]
    PP_H = T * J
    ED_G = NH * H * JH          # ed [g][h][i'][j']
    ED_H = H * JH
    DEN_G = NH * T
    O2_G = T * D
    PR_G = D * T * D            # prod [g][dm][t][e]
    RES_SET = G * T * D

    with ExitStack() as ctx:
        sb = lambda nm, shape, dt=F32: ctx.enter_context(
            nc.sbuf_tensor(nm, shape, dt))
        wsh = sb("wsh", [P, HLEN], F16)
        xin = sb("xin", [P, 2, G, T, D])
        xt16 = sb("xt16", [P, 2, G, D, T], F16)
        xpb = sb("xpb", [P, 2, POS, G, T, JH], F16)
        yv = sb("yv", [P, 2, 2, G, D, J], F16)
        t0 = sb("t0", [P, 2, G, NH, T, J], F16)
        t1 = sb("t1", [P, 2, G, NH, T, J], F16)
        tmp = sb("tmp", [P, G, D, J], F16)
        ed = sb("ed", [P, G, NH, H, JH], F16)
        pp = sb("pp", [P, G, NH, T, J], F16)
        den = sb("den", [P, G, NH, T])
        rcp = sb("rcp", [P, G, NH, T])
        o2 = sb("o2", [P, G, T, D])
        o2b = sb("o2b", [P, G, T, D], F16)
        prod = sb("prod", [P, G, D, T, D], F16)
        res = sb("res", [P, 2, G, T, D])

        sem_names = ["dma_in0", "dma_in1", "const", "xin_done", "prep_done",
                     "s_done", "e_done", "b_done", "res_done", "out0", "out1"]
        sems = {k: ctx.enter_context(nc.semaphore(name=k)) for k in sem_names}

        block = ctx.enter_context(nc.Block())

        @block.gpsimd
        def _(sync):
            # DMA orchestration ONLY (gpsimd tensor ops would starve DVE's
            # SBUF ports). SWDGE: one +16 completion inc per dma_start.
            def store(k):
                sp = k % 2
                sync.wait_ge(sems["res_done"], k + 1)
                sync.dma_start(
                    out=outr[k],
                    in_=_ap(res, sp * RES_SET, [(XIN_G, G), (1, T * D)]),
                ).then_inc(sems["out0" if sp == 0 else "out1"], 16)

            sync.dma_start(out=wsh[:], in_=wth_b).then_inc(sems["const"], 16)
            for n in range(NT):
                s = n % 2
                if n >= 2:
                    sync.wait_ge(sems["xin_done"], n - 1)
                sync.dma_start(
                    out=_ap(xin, s * XIN_SET, [(XIN_G, G), (1, T * D)]),
                    in_=xr[n],
                ).then_inc(sems["dma_in0" if s == 0 else "dma_in1"], 16)
                if n >= 2:
                    store(n - 2)
            store(NT - 2)
            store(NT - 1)
            sync.wait_ge(sems["out0"], 16 * ((NT + 1) // 2))
            sync.wait_ge(sems["out1"], 16 * (NT // 2))

        @block.scalar
        def _(scalar):
            Copy = mybir.ActivationFunctionType.Copy
            Exp = mybir.ActivationFunctionType.Exp

            def prep(n):
                s = n % 2
                scalar.wait_ge(sems["dma_in0" if s == 0 else "dma_in1"],
                               16 * (n // 2 + 1))
                # xt16[g][d][t] = x[g][t][d]
                scalar.activation(
                    out=_ap(xt16, s * XT_SET, [(XT_G, G), (T, D), (1, T)]),
                    in_=_ap(xin, s * XIN_SET, [(XIN_G, G), (1, D), (D, T)]),
                    func=Copy)
                # xpb[a][g][i][j] = xp[g][i][a]  (AB: all i, j<18; C: i,j>=split)
                for a in range(POS):
                    act = scalar.activation(
                        out=_ap(xpb, s * XPB_SET + a * XPB_A,
                                [(XPB_G, G), (JH, T), (1, JH)]),
                        in_=_ap(xin, s * XIN_SET + POS + a,
                                [(XIN_G, G), (D, T), (0, JH)]),
                        func=Copy)
                    if a == POS - 1:
                        act.then_inc(sems["xin_done"], 1)

            def expf(n):
                s = n % 2
                scalar.wait_ge(sems["s_done"], n + 1)
                if n >= 2:
                    scalar.wait_ge(sems["b_done"], n - 1)
                # AB: all i, j<18 ((g,h) merged: TS_G == NH*TS_H)
                scalar.activation(
                    out=_ap(t1, s * TS_SET, [(TS_H, G * NH), (J, T), (1, JH)]),
                    in_=_ap(t0, s * TS_SET, [(TS_H, G * NH), (J, T), (1, JH)]),
                    func=Exp)
                # C: i>=17, j>=18
                scalar.activation(
                    out=_ap(t1, s * TS_SET + H * J + JH,
                            [(TS_H, G * NH), (J, H), (1, JH)]),
                    in_=_ap(t0, s * TS_SET + H * J + JH,
                            [(TS_H, G * NH), (J, H), (1, JH)]),
                    func=Exp,
                ).then_inc(sems["e_done"], 1)

            scalar.wait_ge(sems["const"], 16)
            prep(0)
            prep(1)
            for n in range(NT):
                expf(n)
                if n + 2 < NT:
                    prep(n + 2)

        @block.vector
        def _(vector):
            vector.wait_ge(sems["const"], 16)
            # zero the j=34,35 pad columns of yv once (proj never writes
            # them; PV/scores read them; mask kills their contribution but
            # they must be finite)
            vector.memset(
                _ap(yv, T, [(J, 2 * 2 * G * D), (1, J - T)]), 0.0)

            def phase_a(n):
                s = n % 2
                # xin_done is incremented by ACT prep(n)'s last copy, which
                # also certifies xt16/xpb for this tile
                vector.wait_ge(sems["xin_done"], n + 1)
                # projections: yv[w][g][hc][j] = sum_b xt16[g][3(1-w)+b][j] * A2[w][hc][b]
                for w in range(2):
                    for b in range(POS):
                        i0 = _ap(xt16, s * XT_SET + (3 * (1 - w) + b) * T,
                                 [(XT_G, G), (0, D), (1, T)])
                        i1 = _ap(wsh, OFFH_A2 + (w * POS + b) * D * T,
                                 [(0, G), (T, D), (1, T)])
                        if b == 0:
                            vector.tensor_mul(
                                out=_ap(yv, s * YV_SET + w * YV_W,
                                        [(YV_G, G), (J, D), (1, T)]),
                                in0=i0, in1=i1)
                        else:
                            vector.tensor_mul(
                                out=_ap(tmp, 0, [(D * J, G), (J, D), (1, T)]),
                                in0=i0, in1=i1)
                            vector.tensor_add(
                                out=_ap(yv, s * YV_SET + w * YV_W,
                                        [(YV_G, G), (J, D), (1, T)]),
                                in0=_ap(yv, s * YV_SET + w * YV_W,
                                        [(YV_G, G), (J, D), (1, T)]),
                                in1=_ap(tmp, 0, [(D * J, G), (J, D), (1, T)]))
                # scores (smul split per head: xpb has no h dim to merge)
                def smul(a, dst):
                    for h in range(NH):
                        # AB: all i, j<18
                        vector.tensor_mul(
                            out=_ap(dst, s * TS_SET + h * TS_H,
                                    [(TS_G, G), (J, T), (1, JH)]),
                            in0=_ap(xpb, s * XPB_SET + a * XPB_A,
                                    [(XPB_G, G), (JH, T), (1, JH)]),
                            in1=_ap(yv, s * YV_SET + (h * HD + a) * J,
                                    [(YV_G, G), (0, T), (1, JH)]))
                        # C: i>=17, j>=18 (xpb cols 0:18 hold the same xp)
                        vector.tensor_mul(
                            out=_ap(dst, s * TS_SET + h * TS_H + H * J + JH,
                                    [(TS_G, G), (J, H), (1, JH)]),
                            in0=_ap(xpb, s * XPB_SET + a * XPB_A + H * JH,
                                    [(XPB_G, G), (JH, H), (1, JH)]),
                            in1=_ap(yv, s * YV_SET + (h * HD + a) * J + JH,
                                    [(YV_G, G), (0, H), (1, JH)]))

                def tadd(region):
                    li = T if region == "AB" else H
                    off = 0 if region == "AB" else H * J + JH
                    vector.tensor_add(
                        out=_ap(t0, s * TS_SET + off,
                                [(TS_H, G * NH), (J, li), (1, JH)]),
                        in0=_ap(t0, s * TS_SET + off,
                                [(TS_H, G * NH), (J, li), (1, JH)]),
                        in1=_ap(t1, s * TS_SET + off,
                                [(TS_H, G * NH), (J, li), (1, JH)]))

                smul(0, t0)
                smul(1, t1)
                tadd("AB")
                tadd("C")
                smul(2, t1)
                # mask: t1[i<17, j<18] += maskA ; t1[C] += maskC
                vector.tensor_add(
                    out=_ap(t1, s * TS_SET, [(TS_H, G * NH), (J, H), (1, JH)]),
                    in0=_ap(t1, s * TS_SET, [(TS_H, G * NH), (J, H), (1, JH)]),
                    in1=_ap(wsh, OFFH_MA, [(0, G * NH), (JH, H), (1, JH)]))
                vector.tensor_add(
                    out=_ap(t1, s * TS_SET + H * J + JH,
                            [(TS_H, G * NH), (J, H), (1, JH)]),
                    in0=_ap(t1, s * TS_SET + H * J + JH,
                            [(TS_H, G * NH), (J, H), (1, JH)]),
                    in1=_ap(wsh, OFFH_MC, [(0, G * NH), (JH, H), (1, JH)]))
                tadd("AB")
                vector.tensor_add(
                    out=_ap(t0, s * TS_SET + H * J + JH,
                            [(TS_H, G * NH), (J, H), (1, JH)]),
                    in0=_ap(t0, s * TS_SET + H * J + JH,
                            [(TS_H, G * NH), (J, H), (1, JH)]),
                    in1=_ap(t1, s * TS_SET + H * J + JH,
                            [(TS_H, G * NH), (J, H), (1, JH)])
                ).then_inc(sems["s_done"], 1)

            def fold_chain(tsrc, soff, rows, into_ed):
                """rows i>=17: fold j[18:36) into j[0:18) (into ed or in
                place), then [16:18)->[0:2), [8:16)->[0:8)."""
                pass  # structured inline below for clarity

            def phase_b(n):
                s = n % 2
                vector.wait_ge(sems["e_done"], n + 1)
                if n >= 2:
                    vector.wait_ge(sems["out0" if s == 0 else "out1"],
                                   16 * (n // 2))
                # den: ed = e[i>=17, j<18] + e[i>=17, j>=18]  ((g,h) merged)
                vector.tensor_add(
                    out=_ap(ed, 0, [(ED_H, G * NH), (JH, H), (1, JH)]),
                    in0=_ap(t1, s * TS_SET + H * J,
                            [(TS_H, G * NH), (J, H), (1, JH)]),
                    in1=_ap(t1, s * TS_SET + H * J + JH,
                            [(TS_H, G * NH), (J, H), (1, JH)]))
                # fold [10:18) onto [0:8), then reduce the contiguous [0:10)
                vector.tensor_add(
                    out=_ap(ed, 0, [(ED_H, G * NH), (JH, H), (1, 8)]),
                    in0=_ap(ed, 0, [(ED_H, G * NH), (JH, H), (1, 8)]),
                    in1=_ap(ed, 10, [(ED_H, G * NH), (JH, H), (1, 8)]))
                vector.tensor_reduce(
                    out=_ap(den, 0, [(T, G * NH), (1, H)]),
                    in_=_ap(t1, s * TS_SET,
                            [(TS_H, G * NH), (J, H), (1, JH)]),
                    axis=mybir.AxisListType.X, op=mybir.AluOpType.add)
                vector.tensor_reduce(
                    out=_ap(den, H, [(T, G * NH), (1, H)]),
                    in_=_ap(ed, 0, [(ED_H, G * NH), (JH, H), (1, 10)]),
                    axis=mybir.AxisListType.X, op=mybir.AluOpType.add)
                vector.reciprocal(
                    out=_ap(rcp, 0, [(1, G * NH * T)]),
                    in_=_ap(den, 0, [(1, G * NH * T)]))
                # PV per channel c  (yv (g,h) merge: YV_G == NH*HD*J)
                for c in range(HD):
                    vector.tensor_mul(
                        out=_ap(pp, 0, [(PP_H, G * NH), (J, T), (1, JH)]),
                        in0=_ap(t1, s * TS_SET,
                                [(TS_H, G * NH), (J, T), (1, JH)]),
                        in1=_ap(yv, s * YV_SET + YV_W + c * J,
                                [(HD * J, G * NH), (0, T), (1, JH)]))
                    mm = vector.tensor_mul(
                        out=_ap(pp, H * J + JH,
                                [(PP_H, G * NH), (J, H), (1, JH)]),
                        in0=_ap(t1, s * TS_SET + H * J + JH,
                                [(TS_H, G * NH), (J, H), (1, JH)]),
                        in1=_ap(yv, s * YV_SET + YV_W + c * J + JH,
                                [(HD * J, G * NH), (0, H), (1, JH)]))
                    if c == HD - 1:
                        mm.then_inc(sems["b_done"], 1)
                    vector.tensor_add(
                        out=_ap(pp, H * J, [(PP_H, G * NH), (J, H), (1, JH)]),
                        in0=_ap(pp, H * J, [(PP_H, G * NH), (J, H), (1, JH)]),
                        in1=_ap(pp, H * J + JH,
                                [(PP_H, G * NH), (J, H), (1, JH)]))
                    vector.tensor_add(
                        out=_ap(pp, 0, [(PP_H, G * NH), (J, T), (1, 8)]),
                        in0=_ap(pp, 0, [(PP_H, G * NH), (J, T), (1, 8)]),
                        in1=_ap(pp, 10, [(PP_H, G * NH), (J, T), (1, 8)]))
                    vector.tensor_reduce(
                        out=_ap(o2, c, [(O2_G, G), (HD, NH), (D, T)]),
                        in_=_ap(pp, 0, [(PP_H, G * NH), (J, T), (1, 10)]),
                        axis=mybir.AxisListType.X, op=mybir.AluOpType.add)
                # normalize -> fp16 (split per head: rcp not (h,c)-mergeable)
                for h in range(NH):
                    vector.tensor_mul(
                        out=_ap(o2b, h * HD,
                                [(O2_G, G), (D, T), (1, HD)]),
                        in0=_ap(o2, h * HD,
                                [(O2_G, G), (D, T), (1, HD)]),
                        in1=_ap(rcp, h * T,
                                [(DEN_G, G), (1, T), (0, HD)]))
                # output projection (WoM replicated over t -> (t,e) merge)
                vector.tensor_mul(
                    out=_ap(prod, 0, [(PR_G, G), (T * D, D), (1, T * D)]),
                    in0=_ap(o2b, 0, [(O2_G, G), (0, D), (1, T * D)]),
                    in1=_ap(wsh, OFFH_WO, [(0, G), (T * D, D), (1, T * D)]))
                vector.tensor_reduce(
                    out=_ap(res, s * RES_SET, [(RES_SET // G, G), (1, D), (D, T)]),
                    in_=_ap(prod, 0, [(PR_G, G), (D, D * T), (1, D)]),
                    axis=mybir.AxisListType.X, op=mybir.AluOpType.add
                ).then_inc(sems["res_done"], 1)

            for n in range(NT):
                phase_a(n)
                if n >= 1:
                    phase_b(n - 1)
            phase_b(NT - 1)

    return nc


def _pack_weights(Wq, Wk, Wv, Wo):
    wth = np.zeros(HLEN, dtype=np.float16)
    scale = 1.0 / math.sqrt(HD)
    A2 = np.zeros((2, D, POS), dtype=np.float64)
    for h in range(NH):
        A2[0, h * HD:(h + 1) * HD, :] = (Wq[h * HD:(h + 1) * HD, :].T
                                         @ Wk[h * HD:(h + 1) * HD, :]) * scale
        A2[1, h * HD:(h + 1) * HD, :] = Wv[h * HD:(h + 1) * HD, :]
    # A2 replicated over t: [w][b][hc][t] = A2[w][hc][b]
    a2r = np.broadcast_to(
        A2.astype(np.float16).transpose(0, 2, 1)[:, :, :, None],
        (2, POS, D, T))
    wth[OFFH_A2:OFFH_A2 + 2 * POS * D * T] = a2r.reshape(-1)
    # WoM replicated over t: [dm][t][e] = Wo[dm][e]
    wom = np.broadcast_to(Wo.astype(np.float16)[:, None, :], (D, T, D))
    wth[OFFH_WO:OFFH_WO + D * T * D] = wom.reshape(-1)
    # maskA: rows i<17, cols j<18: -30000 where j > i
    ma = np.zeros((H, JH), dtype=np.float16)
    for i in range(H):
        ma[i, i + 1:] = MASKV
    # maskC: rows i'=i-17, cols j'=j-18: -30000 where j' >= i'  (covers pad)
    mc = np.zeros((H, JH), dtype=np.float16)
    for i in range(H):
        mc[i, i:] = MASKV
    wth[OFFH_MA:OFFH_MA + H * JH] = ma.reshape(-1)
    wth[OFFH_MC:OFFH_MC + H * JH] = mc.reshape(-1)
    return wth


@lru_cache(maxsize=2)
def _cached_kernel(bc, G):
    return build_kernel(bc, G)


def _prepare(x, Wq, Wk, Wv, Wo, G=4):
    x = np.ascontiguousarray(x, dtype=np.float32)
    B = x.shape[0]
    bc = B // NCORES
    nc = _cached_kernel(bc, G)
    wth = _pack_weights(np.asarray(Wq, dtype=np.float32),
                        np.asarray(Wk, dtype=np.float32),
                        np.asarray(Wv, dtype=np.float32),
                        np.asarray(Wo, dtype=np.float32))
    in_maps = [{"x": x[i * bc:(i + 1) * bc], "wth": wth}
               for i in range(NCORES)]
    return nc, in_maps


def kernel(x, Wq, Wk, Wv, Wo):
    nc, in_maps = _prepare(x, Wq, Wk, Wv, Wo)
    r = run_bass_kernel_spmd(nc, in_maps, core_ids=list(range(NCORES)))
    return np.concatenate([m["out"] for m in r.results], axis=0)


# revision 21
# speedup vs baseline: 1.2040x; 1.0222x over previous
"""Trainium2 Bass kernel for nn_Attn_40046275068166.

Tiny causal MHA over huge batch: x[B=65536, T=34, D=6], 2 heads, head_dim 3.
Pure data parallelism over 8 cores; batch on the 128 SBUF partitions inside
each core; per-example compute in the free dims.

v2 design notes (calibrated on HW probes):
- DVE tensor_tensor runs 2x ONLY when every operand is 2-byte, innermost
  stride +-1, 4B-aligned start, EVEN run length. Odd 17-runs degrade to
  ~0.8 ns/elem. So score planes are padded to J=36 columns and split at
  j=18; all fp16 ops use even runs.
- tensor_reduce is always 1x (~1.05 ns/elem) -> fold rows with 2x fp16
  adds (36->18->(8+2)) before reducing.
- GPSIMD tensor ops share SBUF ports with DVE and starve it 4.5x ->
  gpsimd does DMA orchestration ONLY.
- ACT (scalar engine) runs ~0.87 ns/elem on any strides; it materializes
  the xp broadcast planes (xpb) and the x transpose (xt16) so every DVE
  mul is all-fp16 stride-1, and computes exp.
- Weights are compile-time -> folded q/k projection A = Wq^T Wk / sqrt(hd);
  mask additive -30000 (fp16-safe, exp -> 0) covers the causal upper
  triangles AND the j=34,35 pad columns.

Pipeline: Pool streams x in / res out (SWDGE, parity-split semaphores);
ACT preps tile n+1 (xt16/xpb) then exps tile n; DVE does phase_a(n)
(proj+scores) then phase_b(n-1) (den/PV/out-projection).
"""

import math
from contextlib import ExitStack
from functools import lru_cache

import numpy as np

import concourse.bass as bass
from concourse import mybir
from concourse.bass_utils import run_bass_kernel_spmd

NCORES = 8
T = 34
J = 36              # padded score-plane width
H = 17              # i split
JH = 18             # j split
D = 6
NH = 2
HD = 3
POS = 3
P = 128

F32 = mybir.dt.float32
F16 = mybir.dt.float16

# fp16 constants layout (element offsets)
OFFH_MA = 0                 # maskA [17][18]
OFFH_MC = H * JH            # maskC [17][18]
OFFH_WO = 2 * H * JH        # WoM replicated [6][34][6] (WoM[dm][t][e]=Wo[dm][e])
OFFH_A2 = OFFH_WO + D * T * D   # A2 replicated [2][3(b)][6(hc)][34(t)]
HLEN = OFFH_A2 + 2 * POS * D * T

MASKV = -30000.0


def _ap(t, off, dims):
    p0 = t[:].ap[0]
    return bass.AP(tensor=t, offset=off, ap=[list(p0)] + [list(d) for d in dims])


def build_kernel(bc, G):
    assert bc % (P * G) == 0
    NT = bc // (P * G)

    nc = bass.Bass("TRN2")
    x = nc.dram_tensor("x", [bc, T, D], F32, kind="ExternalInput")
    wth = nc.dram_tensor("wth", [HLEN], F16, kind="ExternalInput")
    out = nc.dram_tensor("out", [bc, T, D], F32, kind="ExternalOutput")

    xr = x[:].rearrange("(n g p) t d -> n p g t d", g=G, p=P)
    outr = out[:].rearrange("(n g p) t d -> n p g t d", g=G, p=P)
    wth_b = bass.AP(tensor=wth, offset=0, ap=[[0, P], [1, HLEN]])

    # element strides
    XIN_SET = G * T * D         # 408
    XIN_G = T * D               # 34*6
    XT_SET = G * D * T          # xt16 [set][g][d][t]
    XT_G = D * T
    XPB_SET = POS * G * T * JH  # xpb [set][a][g][i][j<18] (C reuses cols 0:18)
    XPB_A = G * T * JH
    XPB_G = T * JH
    YV_SET = 2 * G * D * J      # yv [set][w][g][hc][j]
    YV_W = G * D * J
    YV_G = D * J
    TS_SET = G * NH * T * J     # t0/t1 [set][g][h][i][j]
    TS_G = NH * T * J
    TS_H = T * J
    PP_G = NH * T * J           # pp [g][h][i][j]
    PP_H = T * J
    ED_G = NH * H * JH          # ed [g][h][i'][j']
    ED_H = H * JH
    DEN_G = NH * T
    O2_G = T * D
    PR_G = D * T * D            # prod [g][dm][t][e]
    RES_SET = G * T * D

    with ExitStack() as ctx:
        sb = lambda nm, shape, dt=F32: ctx.enter_context(
            nc.sbuf_tensor(nm, shape, dt))
        wsh = sb("wsh", [P, HLEN], F16)
        xin = sb("xin", [P, 2, G, T, D])
        xt16 = sb("xt16", [P, 2, G, D, T], F16)
        xpb = sb("xpb", [P, 2, POS, G, T, JH], F16)
        yv = sb("yv", [P, 2, 2, G, D, J], F16)
        t0 = sb("t0", [P, 2, G, NH, T, J], F16)
        t1 = sb("t1", [P, 2, G, NH, T, J], F16)
        tmp = sb("tmp", [P, G, D, J], F16)
        ed = sb("ed", [P, G, NH, H, JH], F16)
        pp = sb("pp", [P, G, NH, T, J], F16)
        den = sb("den", [P, G, NH, T])
        rcp = sb("rcp", [P, G, NH, T])
        o2 = sb("o2", [P, G, T, D])
        o2b = sb("o2b", [P, G, T, D], F16)
        prod = sb("prod", [P, G, D, T, D], F16)
        res = sb("res", [P, 2, G, T, D])

        sem_names = ["dma_in0", "dma_in1", "const", "xin_done", "prep_done",
                     "s_done", "e_done", "b_done", "res_done", "out0", "out1"]
        sems = {k: ctx.enter_context(nc.semaphore(name=k)) for k in sem_names}

        block = ctx.enter_context(nc.Block())

        @block.gpsimd
        def _(sync):
            # DMA orchestration ONLY (gpsimd tensor ops would starve DVE's
            # SBUF ports). SWDGE: one +16 completion inc per dma_start.
            def store(k):
                sp = k % 2
                sync.wait_ge(sems["res_done"], k + 1)
                sync.dma_start(
                    out=outr[k],
                    in_=_ap(res, sp * RES_SET, [(XIN_G, G), (1, T * D)]),
                ).then_inc(sems["out0" if sp == 0 else "out1"], 16)

            sync.dma_start(out=wsh[:], in_=wth_b).then_inc(sems["const"], 16)
            for n in range(NT):
                s = n % 2
                if n >= 2:
                    sync.wait_ge(sems["xin_done"], n - 1)
                sync.dma_start(
                    out=_ap(xin, s * XIN_SET, [(XIN_G, G), (1, T * D)]),
                    in_=xr[n],
                ).then_inc(sems["dma_in0" if s == 0 else "dma_in1"], 16)
                if n >= 2:
                    store(n - 2)
            store(NT - 2)
            store(NT - 1)
            sync.wait_ge(sems["out0"], 16 * ((NT + 1) // 2))
            sync.wait_ge(sems["out1"], 16 * (NT // 2))

        @block.scalar
        def _(scalar):
            Copy = mybir.ActivationFunctionType.Copy
            Exp = mybir.ActivationFunctionType.Exp

            def prep(n):
                s = n % 2
                scalar.wait_ge(sems["dma_in0" if s == 0 else "dma_in1"],
                               16 * (n // 2 + 1))
                # xt16[g][d][t] = x[g][t][d]
                scalar.activation(
                    out=_ap(xt16, s * XT_SET, [(XT_G, G), (T, D), (1, T)]),
                    in_=_ap(xin, s * XIN_SET, [(XIN_G, G), (1, D), (D, T)]),
                    func=Copy)
                # xpb[a][g][i][j] = xp[g][i][a]  (AB: all i, j<18; C: i,j>=split)
                for a in range(POS):
                    act = scalar.activation(
                        out=_ap(xpb, s * XPB_SET + a * XPB_A,
                                [(XPB_G, G), (JH, T), (1, JH)]),
                        in_=_ap(xin, s * XIN_SET + POS + a,
                                [(XIN_G, G), (D, T), (0, JH)]),
                        func=Copy)
                    if a == POS - 1:
                        act.then_inc(sems["xin_done"], 1)

            def expf(n):
                s = n % 2
                scalar.wait_ge(sems["s_done"], n + 1)
                if n >= 2:
                    scalar.wait_ge(sems["b_done"], n - 1)
                # AB: all i, j<18 ((g,h) merged: TS_G == NH*TS_H)
                scalar.activation(
                    out=_ap(t1, s * TS_SET, [(TS_H, G * NH), (J, T), (1, JH)]),
                    in_=_ap(t0, s * TS_SET, [(TS_H, G * NH), (J, T), (1, JH)]),
                    func=Exp)
                # C: i>=17, j>=18
                scalar.activation(
                    out=_ap(t1, s * TS_SET + H * J + JH,
                            [(TS_H, G * NH), (J, H), (1, JH)]),
                    in_=_ap(t0, s * TS_SET + H * J + JH,
                            [(TS_H, G * NH), (J, H), (1, JH)]),
                    func=Exp,
                ).then_inc(sems["e_done"], 1)

            scalar.wait_ge(sems["const"], 16)
            prep(0)
            prep(1)
            for n in range(NT):
                expf(n)
                if n + 2 < NT:
                    prep(n + 2)

        @block.vector
        def _(vector):
            vector.wait_ge(sems["const"], 16)
            # zero the j=34,35 pad columns of yv once (proj never writes
            # them; PV/scores read them; mask kills their contribution but
            # they must be finite)
            vector.memset(
                _ap(yv, T, [(J, 2 * 2 * G * D), (1, J - T)]), 0.0)

            def phase_a(n):
                s = n % 2
                # xin_done is incremented by ACT prep(n)'s last copy, which
                # also certifies xt16/xpb for this tile
                vector.wait_ge(sems["xin_done"], n + 1)
                # projections: yv[w][g][hc][j] = sum_b xt16[g][3(1-w)+b][j] * A2[w][hc][b]
                for w in range(2):
                    for b in range(POS):
                        i0 = _ap(xt16, s * XT_SET + (3 * (1 - w) + b) * T,
                                 [(XT_G, G), (0, D), (1, T)])
                        i1 = _ap(wsh, OFFH_A2 + (w * POS + b) * D * T,
                                 [(0, G), (T, D), (1, T)])
                        if b == 0:
                            vector.tensor_mul(
                                out=_ap(yv, s * YV_SET + w * YV_W,
                                        [(YV_G, G), (J, D), (1, T)]),
                                in0=i0, in1=i1)
                        else:
                            vector.tensor_mul(
                                out=_ap(tmp, 0, [(D * J, G), (J, D), (1, T)]),
                                in0=i0, in1=i1)
                            vector.tensor_add(
                                out=_ap(yv, s * YV_SET + w * YV_W,
                                        [(YV_G, G), (J, D), (1, T)]),
                                in0=_ap(yv, s * YV_SET + w * YV_W,
                                        [(YV_G, G), (J, D), (1, T)]),
                                in1=_ap(tmp, 0, [(D * J, G), (J, D), (1, T)]))
                # scores (smul split per head: xpb has no h dim to merge)
                def smul(a, dst):
                    for h in range(NH):
                        # AB: all i, j<18
                        vector.tensor_mul(
                            out=_ap(dst, s * TS_SET + h * TS_H,
                                    [(TS_G, G), (J, T), (1, JH)]),
                            in0=_ap(xpb, s * XPB_SET + a * XPB_A,
                                    [(XPB_G, G), (JH, T), (1, JH)]),
                            in1=_ap(yv, s * YV_SET + (h * HD + a) * J,
                                    [(YV_G, G), (0, T), (1, JH)]))
                        # C: i>=17, j>=18 (xpb cols 0:18 hold the same xp)
                        vector.tensor_mul(
                            out=_ap(dst, s * TS_SET + h * TS_H + H * J + JH,
                                    [(TS_G, G), (J, H), (1, JH)]),
                            in0=_ap(xpb, s * XPB_SET + a * XPB_A + H * JH,
                                    [(XPB_G, G), (JH, H), (1, JH)]),
                            in1=_ap(yv, s * YV_SET + (h * HD + a) * J + JH,
                                    [(YV_G, G), (0, H), (1, JH)]))

                def tadd(region):
                    li = T if region == "AB" else H
                    off = 0 if region == "AB" else H * J + JH
                    vector.tensor_add(
                        out=_ap(t0, s * TS_SET + off,
                                [(TS_H, G * NH), (J, li), (1, JH)]),
                        in0=_ap(t0, s * TS_SET + off,
                                [(TS_H, G * NH), (J, li), (1, JH)]),
                        in1=_ap(t1, s * TS_SET + off,
                                [(TS_H, G * NH), (J, li), (1, JH)]))

                smul(0, t0)
                smul(1, t1)
                tadd("AB")
                tadd("C")
                smul(2, t1)
                # mask: t1[i<17, j<18] += maskA ; t1[C] += maskC
                vector.tensor_add(
                    out=_ap(t1, s * TS_SET, [(TS_H, G * NH), (J, H), (1, JH)]),
                    in0=_ap(t1, s * TS_SET, [(TS_H, G * NH), (J, H), (1, JH)]),
                    in1=_ap(wsh, OFFH_MA, [(0, G * NH), (JH, H), (1, JH)]))
                vector.tensor_add(
                    out=_ap(t1, s * TS_SET + H * J + JH,
                            [(TS_H, G * NH), (J, H), (1, JH)]),
                    in0=_ap(t1, s * TS_SET + H * J + JH,
                            [(TS_H, G * NH), (J, H), (1, JH)]),
                    in1=_ap(wsh, OFFH_MC, [(0, G * NH), (JH, H), (1, JH)]))
                tadd("AB")
                vector.tensor_add(
                    out=_ap(t0, s * TS_SET + H * J + JH,
                            [(TS_H, G * NH), (J, H), (1, JH)]),
                    in0=_ap(t0, s * TS_SET + H * J + JH,
                            [(TS_H, G * NH), (J, H), (1, JH)]),
                    in1=_ap(t1, s * TS_SET + H * J + JH,
                            [(TS_H, G * NH), (J, H), (1, JH)])
                ).then_inc(sems["s_done"], 1)

            def fold_chain(tsrc, soff, rows, into_ed):
                """rows i>=17: fold j[18:36) into j[0:18) (into ed or in
                place), then [16:18)->[0:2), [8:16)->[0:8)."""
                pass  # structured inline below for clarity

            def phase_b(n):
                s = n % 2
                vector.wait_ge(sems["e_done"], n + 1)
                if n >= 2:
                    vector.wait_ge(sems["out0" if s == 0 else "out1"],
                                   16 * (n // 2))
                # den: ed = e[i>=17, j<18] + e[i>=17, j>=18]  ((g,h) merged)
                vector.tensor_add(
                    out=_ap(ed, 0, [(ED_H, G * NH), (JH, H), (1, JH)]),
                    in0=_ap(t1, s * TS_SET + H * J,
                            [(TS_H, G * NH), (J, H), (1, JH)]),
                    in1=_ap(t1, s * TS_SET + H * J + JH,
                            [(TS_H, G * NH), (J, H), (1, JH)]))
                # fold [10:18) onto [0:8), then reduce the contiguous [0:10)
                vector.tensor_add(
                    out=_ap(ed, 0, [(ED_H, G * NH), (JH, H), (1, 8)]),
                    in0=_ap(ed, 0, [(ED_H, G * NH), (JH, H), (1, 8)]),
                    in1=_ap(ed, 10, [(ED_H, G * NH), (JH, H), (1, 8)]))
                vector.tensor_reduce(
                    out=_ap(den, 0, [(T, G * NH), (1, H)]),
                    in_=_ap(t1, s * TS_SET,
                            [(TS_H, G * NH), (J, H), (1, JH)]),
                    axis=mybir.AxisListType.X, op=mybir.AluOpType.add)
                vector.tensor_reduce(
                    out=_ap(den, H, [(T, G * NH), (1, H)]),
                    in_=_ap(ed, 0, [(ED_H, G * NH), (JH, H), (1, 10)]),
                    axis=mybir.AxisListType.X, op=mybir.AluOpType.add)
                vector.reciprocal(
                    out=_ap(rcp, 0, [(1, G * NH * T)]),
                    in_=_ap(den, 0, [(1, G * NH * T)]))
                # PV per channel c  (yv (g,h) merge: YV_G == NH*HD*J)
                for c in range(HD):
                    vector.tensor_mul(
                        out=_ap(pp, 0, [(PP_H, G * NH), (J, T), (1, JH)]),
                        in0=_ap(t1, s * TS_SET,
                                [(TS_H, G * NH), (J, T), (1, JH)]),
                        in1=_ap(yv, s * YV_SET + YV_W + c * J,
                                [(HD * J, G * NH), (0, T), (1, JH)]))
                    mm = vector.tensor_mul(
                        out=_ap(pp, H * J + JH,
                                [(PP_H, G * NH), (J, H), (1, JH)]),
                        in0=_ap(t1, s * TS_SET + H * J + JH,
                                [(TS_H, G * NH), (J, H), (1, JH)]),
                        in1=_ap(yv, s * YV_SET + YV_W + c * J + JH,
                                [(HD * J, G * NH), (0, H), (1, JH)]))
                    if c == HD - 1:
                        mm.then_inc(sems["b_done"], 1)
                    vector.tensor_add(
                        out=_ap(pp, H * J, [(PP_H, G * NH), (J, H), (1, JH)]),
                        in0=_ap(pp, H * J, [(PP_H, G * NH), (J, H), (1, JH)]),
                        in1=_ap(pp, H * J + JH,
                                [(PP_H, G * NH), (J, H), (1, JH)]))
                    vector.tensor_add(
                        out=_ap(pp, 0, [(PP_H, G * NH), (J, T), (1, 8)]),
                        in0=_ap(pp, 0, [(PP_H, G * NH), (J, T), (1, 8)]),
                        in1=_ap(pp, 10, [(PP_H, G * NH), (J, T), (1, 8)]))
                    vector.tensor_reduce(
                        out=_ap(o2, c * T, [(O2_G, G), (HD * T, NH), (1, T)]),
                        in_=_ap(pp, 0, [(PP_H, G * NH), (J, T), (1, 10)]),
                        axis=mybir.AxisListType.X, op=mybir.AluOpType.add)
                # normalize -> fp16, t-inner layout o2b[g][e][t]
                for h in range(NH):
                    vector.tensor_mul(
                        out=_ap(o2b, h * HD * T,
                                [(O2_G, G), (T, HD), (1, T)]),
                        in0=_ap(o2, h * HD * T,
                                [(O2_G, G), (T, HD), (1, T)]),
                        in1=_ap(rcp, h * T,
                                [(DEN_G, G), (0, HD), (1, T)]))
                # output projection: prod[g][dm][e][t] = o2b[g][e][t]*Wo[dm][e]
                for dm in range(D):
                    vector.tensor_mul(
                        out=_ap(prod, dm * D * T,
                                [(PR_G, G), (T, D), (1, T)]),
                        in0=_ap(o2b, 0, [(O2_G, G), (T, D), (1, T)]),
                        in1=_ap(wsh, OFFH_WO + dm * D * T,
                                [(0, G), (T, D), (1, T)]))
                # fold the 6-term e-sum: e[0:3)+=e[3:6); e0+=e1; res=e0+e2
                vector.tensor_add(
                    out=_ap(prod, 0, [(D * T, G * D), (T, HD), (1, T)]),
                    in0=_ap(prod, 0, [(D * T, G * D), (T, HD), (1, T)]),
                    in1=_ap(prod, HD * T, [(D * T, G * D), (T, HD), (1, T)]))
                vector.tensor_add(
                    out=_ap(prod, 0, [(D * T, G * D), (1, T)]),
                    in0=_ap(prod, 0, [(D * T, G * D), (1, T)]),
                    in1=_ap(prod, T, [(D * T, G * D), (1, T)]))
                vector.tensor_add(
                    out=_ap(res, s * RES_SET, [(XIN_G, G), (1, D), (D, T)]),
                    in0=_ap(prod, 0, [(PR_G, G), (D * T, D), (1, T)]),
                    in1=_ap(prod, 2 * T, [(PR_G, G), (D * T, D), (1, T)])
                ).then_inc(sems["res_done"], 1)

            for n in range(NT):
                phase_a(n)
                if n >= 1:
                    phase_b(n - 1)
            phase_b(NT - 1)

    return nc


def _pack_weights(Wq, Wk, Wv, Wo):
    wth = np.zeros(HLEN, dtype=np.float16)
    scale = 1.0 / math.sqrt(HD)
    A2 = np.zeros((2, D, POS), dtype=np.float64)
    for h in range(NH):
        A2[0, h * HD:(h + 1) * HD, :] = (Wq[h * HD:(h + 1) * HD, :].T
                                         @ Wk[h * HD:(h + 1) * HD, :]) * scale
        A2[1, h * HD:(h + 1) * HD, :] = Wv[h * HD:(h + 1) * HD, :]
    # A2 replicated over t: [w][b][hc][t] = A2[w][hc][b]
    a2r = np.broadcast_to(
        A2.astype(np.float16).transpose(0, 2, 1)[:, :, :, None],
        (2, POS, D, T))
    wth[OFFH_A2:OFFH_A2 + 2 * POS * D * T] = a2r.reshape(-1)
    # WoT replicated over t: [dm][e][t] = Wo[dm][e]
    wom = np.broadcast_to(Wo.astype(np.float16)[:, :, None], (D, D, T))
    wth[OFFH_WO:OFFH_WO + D * T * D] = wom.reshape(-1)
    # maskA: rows i<17, cols j<18: -30000 where j > i
    ma = np.zeros((H, JH), dtype=np.float16)
    for i in range(H):
        ma[i, i + 1:] = MASKV
    # maskC: rows i'=i-17, cols j'=j-18: -30000 where j' >= i'  (covers pad)
    mc = np.zeros((H, JH), dtype=np.float16)
    for i in range(H):
        mc[i, i:] = MASKV
    wth[OFFH_MA:OFFH_MA + H * JH] = ma.reshape(-1)
    wth[OFFH_MC:OFFH_MC + H * JH] = mc.reshape(-1)
    return wth


@lru_cache(maxsize=2)
def _cached_kernel(bc, G):
    return build_kernel(bc, G)


def _prepare(x, Wq, Wk, Wv, Wo, G=4):
    x = np.ascontiguousarray(x, dtype=np.float32)
    B = x.shape[0]
    bc = B // NCORES
    nc = _cached_kernel(bc, G)
    wth = _pack_weights(np.asarray(Wq, dtype=np.float32),
                        np.asarray(Wk, dtype=np.float32),
                        np.asarray(Wv, dtype=np.float32),
                        np.asarray(Wo, dtype=np.float32))
    in_maps = [{"x": x[i * bc:(i + 1) * bc], "wth": wth}
               for i in range(NCORES)]
    return nc, in_maps


def kernel(x, Wq, Wk, Wv, Wo):
    nc, in_maps = _prepare(x, Wq, Wk, Wv, Wo)
    r = run_bass_kernel_spmd(nc, in_maps, core_ids=list(range(NCORES)))
    return np.concatenate([m["out"] for m in r.results], axis=0)
